# revision 1
# baseline (speedup 1.0000x reference)
"""Trainium2 Bass kernel for nn_Attention_47545287967487.

Causal multi-head attention (B=2, S=2048, D=1024, H=16, DH=64) with QK
RMS-norm, distributed over 8 NeuronCores via head tensor-parallelism:
each core owns 2 heads (a 128-column slice of Wq/Wk/Wv and the matching
128-row slice of Wo), computes its partial output projection, and a
ReduceScatter produces each core's 512-row slice of the final output.

Numerics: projections and the output matmul run in float32r (~1e-4),
attention internals (QK^T, softmax, PV) run in bf16. Scores are bounded
(|q.k|/8 <= 8 after RMS-norm) so softmax skips the max-subtraction pass.

Engine plan per core:
 - PE: x@W projections (fp32r, K-tiled), QK^T with the two heads packed
   into array row-groups (tile_position), PV as [v|1]^T @ P so the
   softmax denominator is a free 65th output row, the output
   projection, selector matmuls for the per-head sum-of-squares
   reduction and rstd broadcast, and 128x128 transposes of v into
   [t,d] layout.
 - ACT: exp (softmax), Square, and rstd = exp(-0.5*ln(mean+eps)) --
   all functions live in one pinned ACT table so there are no table
   reloads. The 1/sqrt(DH) score scale is folded into q's rstd.
 - DVE: PSUM->SBUF casts/copies and the q/k normalize multiplies.
 - GPSIMD: partition-broadcast of the softmax denominator reciprocal
   and constant fills (otherwise idle).

The emission order software-pipelines chunks (x-prefetch, then the
previous chunk's attention, this chunk's projections, then the previous
chunk's output projection) so PE/ACT/DVE interleave across chunk
boundaries. build_nc(repeat=N) unrolls the whole pipeline N times in
one NEFF for slope-based device timing.

kernel(**inputs) takes the FULL unsharded inputs and returns the FULL
[2, 2048, 1024] float32 output.
"""

import math
import numpy as np

import concourse.bacc as bacc
import concourse.mybir as mybir
from concourse import tile
from concourse.bass_utils import run_bass_kernel_spmd

import ml_dtypes

BF16 = ml_dtypes.bfloat16

# Problem shape (hardcoded per the harness contract).
B, S, D, DH = 2, 2048, 1024, 64
H = D // DH
N_CORES = 8
HEADS_PER_CORE = H // N_CORES          # 2
DC = HEADS_PER_CORE * DH               # 128 feature columns per core
EPS = 1e-6

SCHUNK = 512                            # s-chunk width
TT = 128                                # t-tile width
KT = D // 128                           # 8 contraction tiles
NCH = S // SCHUNK                       # 4 s-chunks per batch
ROWS = B * S                            # 4096
ROWS_PER_CORE = ROWS // N_CORES         # 512

F32 = mybir.dt.float32
F32R = mybir.dt.float32r
BF = mybir.dt.bfloat16

# All ACT functions this kernel uses (Square, Ln, Exp, Copy) live in the
# 'natural_log_exp_and_others' table. The default table chooser picks the
# first table containing each function, which thrashes between the exp and
# ln tables (~1.3us per reload, dozens of reloads). Pin the chooser to the
# one table that covers everything by emptying the others (positions are
# preserved so act_func_set_id still indexes act_info.json correctly).
_PINNED_ACT_TABLE = "natural_log_exp_and_others"
_orig_get_act_tables = bacc.get_activation_tables


def _pinned_act_tables(arch):
    tables = _orig_get_act_tables(arch)
    return {
        name: (funcs if name == _PINNED_ACT_TABLE else set())
        for name, funcs in tables.items()
    }


bacc.get_activation_tables = _pinned_act_tables


def build_nc(collective=True, stage=3, repeat=1):
    nc = bacc.Bacc("TRN2", target_bir_lowering=False)

    xt_d = nc.dram_tensor("xt", [D, ROWS], F32R, kind="ExternalInput")
    wq_d = nc.dram_tensor("wq", [D, DC], F32R, kind="ExternalInput")
    wk_d = nc.dram_tensor("wk", [D, DC], F32R, kind="ExternalInput")
    wv_d = nc.dram_tensor("wv", [D, DC], F32R, kind="ExternalInput")
    wo_d = nc.dram_tensor("wo", [DC, D], F32R, kind="ExternalInput")
    mask_d = nc.dram_tensor("mask0", [TT, SCHUNK], BF, kind="ExternalInput")
    ident_d = nc.dram_tensor("ident", [128, 128], BF, kind="ExternalInput")
    ident32_d = nc.dram_tensor("ident32", [128, 128], F32, kind="ExternalInput")
    sel2_d = nc.dram_tensor("sel2", [128, 2], F32R, kind="ExternalInput")
    sel2t_d = nc.dram_tensor("sel2t", [2, 128], F32R, kind="ExternalInput")
    if collective:
        out_d = nc.dram_tensor("out", [ROWS_PER_CORE, D], F32, kind="ExternalOutput")
    else:
        # collective-free variant for TimelineSim: write partials straight out
        out_d = nc.dram_tensor("out", [ROWS, D], F32, kind="ExternalOutput")

    from contextlib import ExitStack
    with tile.TileContext(nc) as tc:
        with ExitStack() as ctx:
            consts = ctx.enter_context(tc.tile_pool(name="consts", bufs=1))
            wpool = ctx.enter_context(tc.tile_pool(name="wpool", bufs=1))
            persist = ctx.enter_context(tc.tile_pool(name="persist", bufs=1))
            xcp = ctx.enter_context(tc.tile_pool(name="xc", bufs=3))
            sqp = ctx.enter_context(tc.tile_pool(name="sqp", bufs=3))
            stdp = ctx.enter_context(tc.tile_pool(name="stdp", bufs=6))
            bcp = ctx.enter_context(tc.tile_pool(name="bcp", bufs=6))
            vtp = ctx.enter_context(tc.tile_pool(name="vtp", bufs=3))
            vaugp = ctx.enter_context(tc.tile_pool(name="vaugp", bufs=40))
            rkp = ctx.enter_context(tc.tile_pool(name="rkp", bufs=40))
            stgp = ctx.enter_context(tc.tile_pool(name="stgp", bufs=2))
            pp = ctx.enter_context(tc.tile_pool(name="pp", bufs=8))
            zbp = ctx.enter_context(tc.tile_pool(name="zbp", bufs=6))
            rcp = ctx.enter_context(tc.tile_pool(name="rcp", bufs=6))
            attallp = ctx.enter_context(tc.tile_pool(name="attall", bufs=3))
            outsbp = ctx.enter_context(tc.tile_pool(name="outsb", bufs=8))
            ps_acc = ctx.enter_context(tc.tile_pool(name="ps_acc", bufs=3, space="PSUM"))
            ps_pt = ctx.enter_context(tc.tile_pool(name="ps_pt", bufs=3, space="PSUM"))
            ps_att = ctx.enter_context(tc.tile_pool(name="ps_att", bufs=2, space="PSUM"))
            dram = ctx.enter_context(tc.tile_pool(name="dram", bufs=1, space="DRAM"))

            # ---- weights first (gate the first projections), then consts,
            # wo last (only needed at the first output projection) ----
            # One merged DMA per weight tensor: SBUF [128, KT*DC] where
            # free-column block k holds DRAM rows [128k, 128k+128) (the
            # k-th contraction tile), so lhsT slices stay [K=128, M=DC].
            w_sb = {}
            for wname, wd in (("q", wq_d), ("k", wk_d), ("v", wv_d)):
                t = wpool.tile([128, KT * DC], F32R, name=f"w{wname}")
                nc.sync.dma_start(
                    t[:].rearrange("p (k c) -> p k c", k=KT),
                    wd[:].rearrange("(k p) c -> p k c", p=128))
                for k in range(KT):
                    w_sb[(wname, k)] = t[:, k * DC:(k + 1) * DC]

            sel2_sb = consts.tile([128, 2], F32R, name="sel2_sb")
            nc.sync.dma_start(sel2_sb[:], sel2_d[:])
            sel2t_sb = consts.tile([2, 128], F32R, name="sel2t_sb")
            nc.sync.dma_start(sel2t_sb[:], sel2t_d[:])
            ident_sb = consts.tile([128, 128], BF, name="ident_sb")
            nc.sync.dma_start(ident_sb[:], ident_d[:])
            mask_sb = consts.tile([TT, SCHUNK], BF, name="mask_sb")
            nc.sync.dma_start(mask_sb[:], mask_d[:])
            ident32_sb = consts.tile([128, 128], F32, name="ident32_sb")
            nc.sync.dma_start(ident32_sb[:], ident32_d[:])
            eps_sb = consts.tile([128, 1], F32, name="eps_sb")
            nc.vector.memset(eps_sb[:], EPS)
            wo_sb = wpool.tile([DC, D], F32R, name="wo_sb")
            nc.sync.dma_start(wo_sb[:], wo_d[:])

            if collective:
                partial = dram.tile([ROWS, D], F32, name="partial")
                rs_out = dram.tile([ROWS_PER_CORE, D], F32, name="rs_out")
            else:
                partial = out_d
                rs_out = None

            # per-chunk q (normalized) / k (raw) bf16, feature-major.
            # Separate tiles per chunk so later-chunk writes never
            # false-share dependency tracking with earlier-chunk reads.
            qts = {}    # (b, i) -> [DC, SCHUNK] bf16
            kts = {}    # (b, i) -> [DC, SCHUNK] bf16
            vaug = {}   # (b, j) -> [128, 2*(DH+1)] bf16
            rk = {}     # (b, j) -> [128, 2] f32: rstd_k/8 per t-position

            def sumsq_rstd(acc_psum, b, i, tag, bias_ap):
                rep = rep_box[0]
                """Per-64-row-group mean-square -> rstd [2, 512] in SBUF.

                rstd = exp(-0.5 * ln(mean + eps) + bias_extra), with the
                optional extra bias (e.g. -ln 8) folded into the Exp's bias.
                """
                sq = sqp.tile([DC, SCHUNK], F32R, name=f"sq_{rep}_{tag}_{b}_{i}", tag="sq")
                nc.scalar.activation(sq[:], acc_psum[:],
                                     mybir.ActivationFunctionType.Square)
                sumsq = ps_pt.tile([2, SCHUNK], F32, name=f"ss_{rep}_{tag}_{b}_{i}",
                                   tag="pt")
                nc.tensor.matmul(sumsq[:], sel2_sb[:], sq[:], start=True, stop=True)
                lm = stdp.tile([2, SCHUNK], F32, name=f"lm_{rep}_{tag}_{b}_{i}", tag="std")
                nc.scalar.activation(lm[:], sumsq[:],
                                     mybir.ActivationFunctionType.Ln,
                                     scale=1.0 / DH, bias=eps_sb[:2, :])
                rstd = stdp.tile([2, SCHUNK], F32R, name=f"rstd_{rep}_{tag}_{b}_{i}",
                                 tag="rstd")
                nc.scalar.activation(rstd[:], lm[:],
                                     mybir.ActivationFunctionType.Exp,
                                     scale=-0.5, bias=bias_ap)
                return rstd

            ln8_sb = consts.tile([128, 1], F32, name="ln8_sb")
            nc.vector.memset(ln8_sb[:], -math.log(DH ** 0.5))

            xcs = {}
            at_alls = {}
            rep_box = [0]

            def prefetch_x(b, i):
                rep = rep_box[0]
                col0 = b * S + i * SCHUNK
                # ---- load xT chunk (one merged strided DMA) ----
                xc = xcp.tile([128, KT * SCHUNK], F32R, name=f"x_{rep}_{b}_{i}",
                              tag="xc")
                # one DMA per k-tile: cheaper first-tile latency, and the
                # first projection matmul can start before the rest land
                for k in range(KT):
                    nc.sync.dma_start(
                        xc[:, k * SCHUNK:(k + 1) * SCHUNK],
                        xt_d[k * 128:(k + 1) * 128, col0:col0 + SCHUNK])
                xcs[(b, i)] = xc

            def proj_q(b, i, xch):
                rep = rep_box[0]
                psq = ps_acc.tile([DC, SCHUNK], F32, name=f"pq_{rep}_{b}_{i}", tag="acc")
                for k in range(KT):
                    nc.tensor.matmul(psq[:], w_sb[("q", k)][:], xch[k][:],
                                     start=(k == 0), stop=(k == KT - 1))
                qtile = persist.tile([DC, SCHUNK], BF, name=f"qt_{rep}_{b}_{i}",
                                     tag="qtk", bufs=20)
                qts[(b, i)] = qtile
                rstd_q = sumsq_rstd(psq, b, i, "q", ln8_sb[:2, :])
                bcq = ps_pt.tile([DC, SCHUNK], F32, name=f"bcq_{rep}_{b}_{i}", tag="pt")
                nc.tensor.matmul(bcq[:], sel2t_sb[:], rstd_q[:],
                                 start=True, stop=True)
                bcqs = bcp.tile([DC, SCHUNK], F32, name=f"bcqs_{rep}_{b}_{i}", tag="bc")
                nc.vector.tensor_copy(bcqs[:], bcq[:])
                nc.vector.tensor_mul(qtile[:], psq[:], bcqs[:])

            def proj_k(b, i, xch):
                rep = rep_box[0]
                psk = ps_acc.tile([DC, SCHUNK], F32, name=f"pk_{rep}_{b}_{i}", tag="acc")
                for k in range(KT):
                    nc.tensor.matmul(psk[:], w_sb[("k", k)][:], xch[k][:],
                                     start=(k == 0), stop=(k == KT - 1))
                ktile = persist.tile([DC, SCHUNK], BF, name=f"kt_{rep}_{b}_{i}",
                                     tag="qtk", bufs=20)
                kts[(b, i)] = ktile
                rstd_k = sumsq_rstd(psk, b, i, "k", 0.0)
                bck = ps_pt.tile([DC, SCHUNK], F32, name=f"bck_{rep}_{b}_{i}", tag="pt")
                nc.tensor.matmul(bck[:], sel2t_sb[:], rstd_k[:],
                                 start=True, stop=True)
                bcks = bcp.tile([DC, SCHUNK], F32, name=f"bcks_{rep}_{b}_{i}", tag="bc")
                nc.vector.tensor_copy(bcks[:], bck[:])
                nc.vector.tensor_mul(ktile[:], psk[:], bcks[:])

            def proj_v(b, i, xch):
                rep = rep_box[0]
                psv = ps_acc.tile([DC, SCHUNK], F32, name=f"pv_{rep}_{b}_{i}", tag="acc")
                for k in range(KT):
                    nc.tensor.matmul(psv[:], w_sb[("v", k)][:], xch[k][:],
                                     start=(k == 0), stop=(k == KT - 1))
                vt = vtp.tile([DC, SCHUNK], BF, name=f"vt_{rep}_{b}_{i}", tag="vt")
                nc.vector.tensor_copy(vt[:], psv[:])
                for u in range(SCHUNK // TT):
                    j = i * (SCHUNK // TT) + u
                    tp = ps_pt.tile([128, 128], BF, name=f"tp_{rep}_{b}_{j}", tag="pt")
                    nc.tensor.transpose(tp[:], vt[:, u * 128:(u + 1) * 128],
                                        ident_sb[:])
                    va = vaugp.tile([128, 2 * (DH + 1)], BF,
                                    name=f"va_{rep}_{b}_{j}", tag="vaug")
                    nc.vector.tensor_copy(
                        va[:].rearrange("p (g d) -> p g d", g=2)[:, :, 0:DH],
                        tp[:].rearrange("p (g d) -> p g d", g=2))
                    nc.gpsimd.memset(
                        va[:].rearrange("p (g d) -> p g d", g=2)[:, :, DH:DH + 1],
                        1.0)
                    vaug[(b, j)] = va

            def proj_parts(b, i):
                xc = xcs.pop((b, i))
                xch = [xc[:, k * SCHUNK:(k + 1) * SCHUNK] for k in range(KT)]
                return [lambda: proj_q(b, i, xch),
                        lambda: proj_k(b, i, xch),
                        lambda: proj_v(b, i, xch)]

            def do_proj(b, i):
                for part in proj_parts(b, i):
                    part()

            def do_attn(b, i, weave=None):
                rep = rep_box[0]
                att = [ps_att.tile([DH + 1, SCHUNK], F32,
                                   name=f"att_{rep}_{b}_{i}_{h}", tag="att")
                       for h in range(HEADS_PER_CORE)]
                n_t = 4 * i + 4
                weave_at = {}
                if weave:
                    for w_idx, part in enumerate(weave):
                        weave_at[1 + w_idx * max(1, (n_t - 1) // len(weave))] = part
                for j in range(n_t):
                    if j in weave_at:
                        weave_at.pop(j)()
                    off = max(0, TT * (j - 4 * i))
                    npx = SCHUNK - off
                    jc, ju = j // 4, j % 4
                    pts = []
                    for h in range(HEADS_PER_CORE):
                        pt = ps_pt.tile([128, SCHUNK], F32,
                                        name=f"ptile_{rep}_{b}_{i}_{j}_{h}", tag="pt")
                        nc.tensor.matmul(
                            pt[:, :npx],
                            kts[(b, jc)][h * DH:(h + 1) * DH,
                                         ju * TT:(ju + 1) * TT],
                            qts[(b, i)][h * DH:(h + 1) * DH, off:SCHUNK],
                            start=True, stop=True,
                            tile_position=(h * DH, 0),
                        )
                        pts.append(pt)
                    for h in range(HEADS_PER_CORE):
                        psb = pp.tile([128, SCHUNK], BF,
                                      name=f"p_{rep}_{b}_{i}_{j}_{h}", tag="p")
                        nc.scalar.activation(psb[:, :npx], pts[h][:, :npx],
                                             mybir.ActivationFunctionType.Exp)
                        if j >= 4 * i:
                            nc.vector.tensor_mul(psb[:, :npx], psb[:, :npx],
                                                 mask_sb[:, :npx])
                        nc.tensor.matmul(
                            att[h][:, off:SCHUNK],
                            vaug[(b, j)][:, h * (DH + 1):(h + 1) * (DH + 1)],
                            psb[:, :npx],
                            start=(j == 0), stop=(j == n_t - 1),
                        )

                for part in list(weave_at.values()):
                    part()

                # ---- normalize by softmax denominator ----
                at_all = attallp.tile([DC, SCHUNK], F32R,
                                      name=f"atall_{rep}_{b}_{i}", tag="attall")
                for h in range(HEADS_PER_CORE):
                    rc = rcp.tile([1, SCHUNK], F32, name=f"rc_{rep}_{b}_{i}_{h}",
                                  tag="rc")
                    nc.vector.reciprocal(rc[:], att[h][DH:DH + 1, :])
                    zbs = zbp.tile([DH, SCHUNK], F32, name=f"zbs_{rep}_{b}_{i}_{h}",
                                   tag="zb")
                    nc.gpsimd.partition_broadcast(zbs[:], rc[:])
                    nc.vector.tensor_mul(at_all[h * DH:(h + 1) * DH, :],
                                         att[h][0:DH, :], zbs[:])

                at_alls[(b, i)] = at_all

            def final_u(b, i, u, at_all):
                rep = rep_box[0]
                # ---- partial output projection (fp32r), one 128-row slab ----
                for n in range(D // 512):
                    op = ps_pt.tile([128, 512], F32,
                                    name=f"op_{rep}_{b}_{i}_{u}_{n}", tag="pt")
                    nc.tensor.matmul(op[:],
                                     at_all[:, u * 128:(u + 1) * 128],
                                     wo_sb[:, n * 512:(n + 1) * 512],
                                     start=True, stop=True)
                    osb = outsbp.tile([128, 512], F32,
                                      name=f"osb_{rep}_{b}_{i}_{u}_{n}",
                                      tag="outsb")
                    nc.vector.tensor_copy(osb[:], op[:])
                    r0 = b * S + i * SCHUNK + u * 128
                    nc.sync.dma_start(
                        partial[r0:r0 + 128, n * 512:(n + 1) * 512],
                        osb[:])

            def final_parts(b, i):
                at_all = at_alls.pop((b, i))
                return [(lambda u=u: final_u(b, i, u, at_all))
                        for u in range(SCHUNK // 128)]

            def do_final(b, i):
                for part in final_parts(b, i):
                    part()


            # Software pipeline. Per step: prefetch x for chunk ci (so its
            # DMA issues ahead of the previous chunk's output stores),
            # attention for chunk ci-1 (its inputs are ready, filling PE/ACT
            # while the x DMA streams), then projections for chunk ci.
            chunks = [(b, i) for b in range(B) for i in range(NCH)]
            for rep_i in range(repeat):
                rep_box[0] = rep_i
                fparts = {}
                for ci in range(len(chunks) + 1):
                    if ci == 0:
                        prefetch_x(*chunks[0])
                        prefetch_x(*chunks[1])
                    elif ci + 1 < len(chunks):
                        prefetch_x(*chunks[ci + 1])
                    if ci >= 1 and stage >= 2:
                        do_attn(*chunks[ci - 1],
                                weave=fparts.pop(ci - 2, None))
                    if ci < len(chunks):
                        do_proj(*chunks[ci])
                    if ci >= 1 and stage >= 3:
                        fparts[ci - 1] = final_parts(*chunks[ci - 1])
                for parts in fparts.values():
                    for part in parts:
                        part()
            if stage == 1:
                # flush qt/kt so the pipeline isn't dead code
                for (b, i), t in qts.items():
                    r0 = b * S + i * SCHUNK
                    nc.sync.dma_start(
                        partial[r0:r0 + 128, 0:256],
                        t[0:128, :].bitcast(F32))
                for (b, i), t in kts.items():
                    r0 = b * S + i * SCHUNK
                    nc.sync.dma_start(
                        partial[r0:r0 + 128, 256:512],
                        t[0:128, :].bitcast(F32))
            elif stage == 2:
                for (b, i), t in list(at_alls.items()):
                    r0 = b * S + i * SCHUNK
                    nc.sync.dma_start(partial[r0:r0 + 128, 0:512],
                                      t[:].bitcast(F32))
            # ---- ReduceScatter partial outputs across the 8 cores ----
            if collective:
                nc.gpsimd.collective_compute(
                    "ReduceScatter",
                    mybir.AluOpType.add,
                    replica_groups=[list(range(N_CORES))],
                    ins=[partial[:]],
                    outs=[rs_out[:]],
                )
                nc.sync.dma_start(out_d[:], rs_out[:])

    nc.compile()
    return nc


_NC_CACHE = {}


def _get_nc():
    if "nc" not in _NC_CACHE:
        _NC_CACHE["nc"] = build_nc()
    return _NC_CACHE["nc"]


def _host_inputs(x, Wq, Wk, Wv, Wo):
    xt = np.ascontiguousarray(x.reshape(ROWS, D).T).astype(np.float32)
    mask0 = (np.arange(TT)[:, None] <= np.arange(SCHUNK)[None, :]).astype(BF16)
    ident = np.eye(128, dtype=BF16)
    ident32 = np.eye(128, dtype=np.float32)
    sel2 = np.zeros((128, 2), dtype=np.float32)
    sel2[:DH, 0] = 1.0
    sel2[DH:2 * DH, 1] = 1.0
    sel2t = np.ascontiguousarray(sel2.T)

    in_maps = []
    for c in range(N_CORES):
        cs = c * DC
        in_maps.append({
            "xt": xt,
            "wq": np.ascontiguousarray(Wq[:, cs:cs + DC]).astype(np.float32),
            "wk": np.ascontiguousarray(Wk[:, cs:cs + DC]).astype(np.float32),
            "wv": np.ascontiguousarray(Wv[:, cs:cs + DC]).astype(np.float32),
            "wo": np.ascontiguousarray(Wo[cs:cs + DC, :]).astype(np.float32),
            "mask0": mask0,
            "ident": ident,
            "ident32": ident32,
            "sel2": sel2,
            "sel2t": sel2t,
        })
    return in_maps


def kernel(x, Wq, Wk, Wv, Wo, mask):
    x = np.asarray(x, dtype=np.float32)
    nc = _get_nc()
    in_maps = _host_inputs(x, np.asarray(Wq), np.asarray(Wk),
                           np.asarray(Wv), np.asarray(Wo))
    res = run_bass_kernel_spmd(nc, in_maps, list(range(N_CORES)))
    full = np.concatenate([res.results[c]["out"] for c in range(N_CORES)], axis=0)
    return full.reshape(B, S, D)


if __name__ == "__main__":
    nc = build_nc()
    print("kernel built and compiled OK")

